# revision 6
# baseline (speedup 1.0000x reference)
"""Trainium2 Bass kernel for the DeepHit-style survival loss.

Math (derived from the reference):
  For each sample i with duration d, event e (u = e>0, st = clip(e-1,0,3)):
    r[k]   = 1 - s[k],  s[k] = sum_c phi[i,c,k]
    lse[k] = log(sum_c e^{phi[i,c,k]} + e^{r[k]})
    loss_i = sum_{k<=d} lse[k] + sum_{k<=d-u} s[k] - u*phi[i,st,d] + (u - d - 1)
  output = mean_i loss_i

Key optimizations:
  - Only columns k <= d_i of sample i contribute, so the host sorts
    samples by d and packs variable-width octets (width = max d in the
    octet + 1, ~half of K on average). Sorted octets are snake-assigned
    to cores so all cores share one width list (one SPMD program) and a
    balanced load. Octet processing order ends on the smallest octet to
    shorten the pipeline drain.
  - Both masked sums run as ONE scalar_tensor_tensor per tile: the
    threshold D = 2d+1-u masks interleaved iotas (2k+1 <= D iff
    k <= d-u, 2k <= D iff k <= d), read via a strided AP over a single
    PSUM tile that holds s and lse regions 512 columns apart.
  - The u*phi[st,d] gather and sum(u-d-1) are exact f64 host terms.

Device mapping per core (8 octets of 8 tiles; tile = 128 samples on
partitions; per-octet width W; PSUM tile of 4 banks per octet laid out
[s chunk0 | se chunk0 | s chunk1 | se chunk1] at 512-col offsets so
each accumulation group sits in its own bank):
  - phi f16 (host cast; tolerance 2e-2), packed [p, tile, cause, k<W],
    DMAed per 4-tile chunk, partition lines contiguous in HBM
  - PE: s = sum_c phi_c via f16 identity matmuls; se = sum_c e^phi
    accumulated likewise + er added last; p-state warmup dummies
  - ACT: exp per chunk (f16 in/out); er = e^(1-s) (bf16 for range,
    fused scale=-1 bias=1) and lse = ln(se) per octet via strided APs
  - DVE: one interleaved masked-sum STT per tile with accum_out
  - host: f64 sum of partials + exact terms from the f32 input
"""

import os
import sys
import numpy as np

for _p in ("/opt/trn_rl_repo",):
    if _p not in sys.path:
        sys.path.insert(0, _p)

import concourse.bass as bass
import concourse.bacc as bacc
import concourse.tile as tile
from concourse import mybir
from concourse.bass_utils import run_bass_kernel_spmd

N_CORES = 8
N, QCAUSE, K = 65536, 4, 128
S = N // N_CORES          # samples per core = 8192
T = S // 128              # tiles (128 samples each) per core = 64
NOCT = T // 8             # 8 octets of 8 tiles per core

F32 = mybir.dt.float32
F16 = mybir.dt.float16
BF16 = mybir.dt.bfloat16

N_PE_WARM = 42

# processing order of the width-ascending rounds: start small (fast
# pipeline fill), end smallest (short drain), biggest in the middle
ORDER = (1, 2, 3, 4, 5, 6, 7, 0)

_CACHE = {}
_LAST = None


def _build_program(widths):
    """widths: per-octet k-widths in PROCESSING order (multiples of 4)."""
    from contextlib import ExitStack

    nc = bacc.Bacc("TRN2", target_bir_lowering=False, debug=False)

    tot = sum(widths)
    # phi packed per partition: per octet [8 tiles x 4 causes x W]
    phi_d = nc.dram_tensor("phi", [128, 32 * tot], F16, kind="ExternalInput").ap()
    # threshold table D = 2d+1-u per (partition, tile)
    cp32_d = nc.dram_tensor("cp32", [128, T], F32, kind="ExternalInput").ap()
    out_d = nc.dram_tensor("acc", [128, T], F32, kind="ExternalOutput").ap()

    # constants: interleaved iota rows [2k+1 | 2k] (f16) and f16 identity
    k_ar = np.arange(K, dtype=np.float16)
    iota_il = np.tile(
        np.concatenate([2 * k_ar + 1, 2 * k_ar]), (128, 1)
    ).astype(np.float16)                                                # [128,256]
    ident_h = np.eye(128, dtype=np.float16)
    cpack16 = np.concatenate(
        [iota_il.view(np.uint16), ident_h.view(np.uint16)], axis=1
    )                                                                   # [128,384]
    cp16_d = nc.inline_tensor(cpack16, name="cp16").ap()

    is_le = mybir.AluOpType.is_le
    mult = mybir.AluOpType.mult
    Exp = mybir.ActivationFunctionType.Exp
    Log = mybir.ActivationFunctionType.Ln

    offs = [0]
    for w in widths:
        offs.append(offs[-1] + 32 * w)

    with tile.TileContext(nc) as tc, ExitStack() as ctx:
        singles = ctx.enter_context(tc.tile_pool(name="singles", bufs=1))
        phip = ctx.enter_context(tc.tile_pool(name="phip", bufs=10))
        octp = ctx.enter_context(tc.tile_pool(name="octp", bufs=4))
        erp = ctx.enter_context(tc.tile_pool(name="erp", bufs=3))
        junkp = ctx.enter_context(tc.tile_pool(name="junkp", bufs=8))
        psp = ctx.enter_context(tc.tile_pool(name="psB", bufs=2, space="PSUM"))

        phiC = {}
        expC = {}
        erB = {}
        psB = {}

        def dma(o, h):
            W = widths[o]
            t = phip.tile([128, 4, 4 * W], F16, tag="phi")
            src = phi_d[:, offs[o] + h * 16 * W : offs[o] + (h + 1) * 16 * W]
            nc.sync.dma_start(out=t, in_=src.rearrange("p (t r) -> p t r", t=4))
            phiC[(o, h)] = t

        def dma_all(o):
            dma(o, 0)
            dma(o, 1)

        def exp_(o, h):
            W = widths[o]
            e = octp.tile([128, 16 * W], F16, tag="exp")
            nc.scalar.activation(
                e, phiC[(o, h)].rearrange("p t r -> p (t r)"), Exp
            )
            expC[(o, h)] = e

        # PSUM layout per octet (one [128, 2048] f32 tile = 4 banks):
        #   bank 2h   = s  of chunk h  (cols 1024h      .. +4W)
        #   bank 2h+1 = se of chunk h  (cols 1024h+512  .. +4W)
        # each accumulation group lives in exactly one bank
        def smm(o, h):
            W = widths[o]
            if h == 0:
                psB[o] = psp.tile([128, 2048], F32, tag="ps", name=f"psB{o}")
            ps = psB[o]
            for c in range(4):
                rhs = phiC[(o, h)][:, :, c * W : (c + 1) * W]
                nc.tensor.matmul(
                    ps[:, 1024 * h : 1024 * h + 4 * W],
                    idh,
                    rhs,
                    start=(c == 0),
                    stop=(c == 3),
                )

        def er_(o):
            W = widths[o]
            e = erp.tile([128, 8 * W], BF16, tag="er")
            src = psB[o].rearrange("p (h s x) -> p h s x", h=2, s=2)[:, :, 0, : 4 * W]
            nc.scalar.activation(
                e.rearrange("p (h x) -> p h x", h=2), src, Exp, bias=1.0, scale=-1.0
            )
            erB[o] = e

        def emm(o, h):
            W = widths[o]
            ps = psB[o]
            eo = expC[(o, h)].rearrange("p (t r) -> p t r", t=4)
            for c in range(4):
                nc.tensor.matmul(
                    ps[:, 1024 * h + 512 : 1024 * h + 512 + 4 * W],
                    idh,
                    eo[:, :, c * W : (c + 1) * W],
                    start=(c == 0),
                    stop=False,
                )

        def er_add(o):
            W = widths[o]
            for h in range(2):
                nc.tensor.matmul(
                    psB[o][:, 1024 * h + 512 : 1024 * h + 512 + 4 * W],
                    idh,
                    erB[o][:, 4 * W * h : 4 * W * (h + 1)],
                    start=False,
                    stop=True,
                )

        def ln_(o):
            W = widths[o]
            ps = psB[o].rearrange("p (h s x) -> p h s x", h=2, s=2)[:, :, 1, : 4 * W]
            nc.scalar.activation(ps, ps, Log)

        def j12(o):
            # one interleaved masked sum per tile:
            #   acc[t] = sum_{k<=d-u} s[k] + sum_{k<=d} lse[k]
            # in1 = [s col | lse col] pair via stride-512 AP; in0 = the
            # matching [2k+1 | 2k] iota pair; threshold D = 2d+1-u
            W = widths[o]
            v4 = psB[o].rearrange("p (q x) -> p q x", x=512)  # [128, 4, 512]
            for ti in range(8):
                t = o * 8 + ti
                h = ti // 4
                col = (ti % 4) * W
                jk = junkp.tile([128, 2, K], F32, tag="j12")
                nc.vector.scalar_tensor_tensor(
                    out=jk[:, :, :W],
                    in0=ioril[:, :, :W],
                    scalar=dthr[:, t : t + 1],
                    in1=v4[:, 2 * h : 2 * h + 2, col : col + W],
                    op0=is_le,
                    op1=mult,
                    accum_out=acc[:, t : t + 1],
                )

        # --- prologue ---
        wdm = singles.tile([128, 128], F16)
        nc.vector.memset(wdm, 1.0)

        dma(0, 0)

        cp32 = singles.tile([128, T], F32)
        nc.sync.dma_start(out=cp32, in_=cp32_d)

        dma(0, 1)
        dma_all(1)

        cp16 = singles.tile([128, 3 * K], mybir.dt.uint16)
        nc.sync.dma_start(out=cp16, in_=cp16_d)
        ioril = cp16[:, : 2 * K].bitcast(F16).rearrange("p (s k) -> p s k", s=2)
        idh = cp16[:, 2 * K :].bitcast(F16)
        dthr = cp32

        acc = singles.tile([128, T], F32)

        # one-time DVE reads of the constants
        warm = singles.tile([128, 2 * K], F16)
        nc.vector.tensor_copy(warm.rearrange("p (s k) -> p s k", s=2), ioril)
        warm2 = singles.tile([128, 1], F32)
        nc.vector.tensor_copy(warm2, dthr[:, 0:1])

        # PE p-state warmup: dummies write an unused corner of the first
        # PSUM tile before its real accumulation groups open
        psB[0] = psp.tile([128, 2048], F32, tag="ps", name="psB0")
        for _ in range(N_PE_WARM):
            nc.tensor.matmul(
                psB[0][:, 1536:1664], wdm, wdm, start=True, stop=True
            )

        dma_all(2)
        dma_all(3)
        exp_(0, 0)
        smm(0, 0)
        exp_(0, 1)
        smm(0, 1)
        er_(0)

        # --- software-pipelined steady state ---
        for o in range(NOCT):
            if o + 4 < NOCT:
                dma_all(o + 4)
            if o > 0:
                j12(o - 1)
            emm(o, 0)
            emm(o, 1)
            er_add(o)
            ln_(o)
            if o + 1 < NOCT:
                exp_(o + 1, 0)
                smm(o + 1, 0)
                exp_(o + 1, 1)
                smm(o + 1, 1)
                er_(o + 1)
        j12(NOCT - 1)

        nc.sync.dma_start(out=out_d, in_=acc)

    # Both Exp and Ln live in the "natural_log_exp_and_others" ACT table
    # set; restrict the registry (preserving set indices) so the
    # table-load pass emits a single hoisted load instead of thrashing.
    import concourse.bacc as _bacc_mod

    real_get = _bacc_mod.get_activation_tables

    def _only_combined(arch):
        tabs = real_get(arch)
        return {
            name: (fns if name == "natural_log_exp_and_others" else set())
            for name, fns in tabs.items()
        }

    _bacc_mod.get_activation_tables = _only_combined
    try:
        nc.finalize()
    finally:
        _bacc_mod.get_activation_tables = real_get
    return nc


def _get_program(widths=None):
    global _LAST
    if widths is None:
        assert _LAST is not None, "call kernel() first"
        return _CACHE[_LAST]
    widths = tuple(widths)
    if widths not in _CACHE:
        _CACHE[widths] = _build_program(widths)
    _LAST = widths
    return _CACHE[widths]


def _widths_sorted(d_s):
    gmax = d_s.reshape(N // 1024, 1024).max(axis=1)
    out = []
    for r in range(NOCT):
        w = int(gmax[8 * r : 8 * r + 8].max()) + 1
        out.append(max(8, (w + 3) // 4 * 4))
    return out


def kernel(phi, idx_durations, events):
    phi = np.asarray(phi)
    d = np.asarray(idx_durations).astype(np.int64)
    e = np.asarray(events).astype(np.int64)
    u = (e > 0).astype(np.int64)
    st = np.clip(e - 1, 0, QCAUSE - 1)

    # exact host-side terms (f64, from the original f32 input)
    phi_flat = np.ascontiguousarray(phi, dtype=np.float32).reshape(N, QCAUSE * K)
    gath = phi_flat[np.arange(N), st * K + d]
    host_term = float((u - d - 1).sum()) - float((gath * u).astype(np.float64).sum())

    # sort samples by duration; octet widths come from the sorted order
    perm = np.argsort(d, kind="stable")
    d_s = d[perm]
    u_s = u[perm]
    phi16_s = phi_flat.astype(np.float16)[perm].reshape(N, QCAUSE, K)

    wsort = _widths_sorted(d_s)
    widths = tuple(wsort[r] for r in ORDER)
    nc = _get_program(widths)

    in_maps = []
    for c in range(N_CORES):
        bufs = []
        thr = np.empty((128, T), dtype=np.float32)
        for j, r in enumerate(ORDER):
            g = 8 * r + (c if r % 2 == 0 else 7 - c)
            W = widths[j]
            base = 1024 * g
            block = phi16_s[base : base + 1024].reshape(8, 128, QCAUSE, K)[
                :, :, :, :W
            ]
            bufs.append(
                np.ascontiguousarray(block.transpose(1, 0, 2, 3)).reshape(128, -1)
            )
            dloc = d_s[base : base + 1024].reshape(8, 128).T  # [128, 8 tiles]
            uloc = u_s[base : base + 1024].reshape(8, 128).T
            thr[:, 8 * j : 8 * j + 8] = 2 * dloc + 1 - uloc
        in_maps.append(
            {
                "phi": np.ascontiguousarray(np.concatenate(bufs, axis=1)),
                "cp32": np.ascontiguousarray(thr),
            }
        )

    trace = os.environ.get("BASS_PROFILE") == "1"
    kw = {}
    if trace:
        tmpdir = os.environ.get("BASS_TRACE_DIR") or None
        kw = dict(trace=True, tmpdir=tmpdir)
    res = run_bass_kernel_spmd(nc, in_maps, list(range(N_CORES)), **kw)
    if trace and res.exec_time_ns is not None:
        print(f"HW exec time: {res.exec_time_ns} ns", file=sys.stderr)

    total = 0.0
    for c in range(N_CORES):
        total += np.asarray(res.results[c]["acc"], dtype=np.float64).sum()
    total += host_term
    return np.float32(total / N)


if __name__ == "__main__":
    rng = np.random.default_rng(0)
    phi = rng.standard_normal((N, QCAUSE, K), dtype=np.float32)
    d = rng.integers(0, K, size=(N,)).astype(np.int64)
    e = rng.integers(0, QCAUSE + 1, size=(N,)).astype(np.int64)
    print(kernel(phi, d, e))
